# revision 1
# baseline (speedup 1.0000x reference)
"""Trainium2 Bass kernel for CropSplit (SipMask-style crop + quadrant split).

Reference computation, per output pixel (y, x, n):
    inside = point (x, y) lies in box rois[n] = (x1, y1, x2, y2)
    cell   = which of the 2x2 ROI sub-cells the pixel falls in
    out[y, x, n] = inside ? data[cell, y, x, n] : 0

Strategy:
  - Shard along W across the 8 cores (25 columns each). Each output pixel is
    independent, so any spatial shard works; W-sharding with an
    [h -> partitions, (w, n) -> free] tile layout makes every DMA row a
    large CONTIGUOUS DRAM block (w,n are the two innermost axes), which is
    what the DMA engines and HBM want. (H-sharding was measured at only
    ~16 GB/s per SDMA engine: 800B chunks with 160KB strides.)
  - The roi-derived masks are computed on host in float32 with
    bit-identical arithmetic to the reference and shipped as uint8:
        hx[w, n]      = (cx == 1)            quadrant column select
        hy[h, n]      = (cy == 1)            quadrant row select
        nin[h, w, n]  = NOT inside(h, w, n)  outer-OR of the two box masks
    (hx is shipped pre-broadcast across partitions; a DMA
    partition-broadcast was measured much slower than a plain load.)
  - Per tile, the 4-way select + mask is 3 predicated DVE ops (cost of a
    DVE op depends only on the free-dim size, so the two x-blends run as
    one op over the plane-pair axis):
        cp(dall[0::2], hx, dall[1::2])  (d0<-d1, d2<-d3 where cx==1)
        cp(dall[0], hy, dall[2])        (y-blend -> 4-way select)
        cp(dall[0], nin, 0)             (zero outside the box)
    All 4 planes of a tile arrive in ONE DMA (single semaphore lane ->
    fewer event-semaphore stalls on the DVE; ~4us faster than 4 loads).
  - h-chunk 2 (rows 128..199) is DMA'd into partitions 28..100 so its
    transfers spread across both SDMA engine groups; DVE ops always run on
    all 128 partitions (free-dim-priced) and out-of-window partitions
    compute garbage that is never stored.
  - DMA issue is split across both HWDGE sequencers (Sync for data loads,
    Scalar for masks/stores); w-blocks are sized small-first for pipeline
    ramp, small-last for tail drain, 6-deep tile buffering in between.
"""

import numpy as np

C = 2
CC = C * C
H = W = N = 200
NCORES = 8
WS = W // NCORES  # 25 columns per core

# (h0, ph, p_off): h rows [h0, h0+ph) live at partitions [p_off, p_off+ph).
# Chunk 2 (72 rows) is DMA'd into partitions 28..100 so its transfers are
# split evenly across both SDMA engine groups (partitions <64 / >=64).
# DVE ops always run on all 128 partitions (cost depends only on the free
# dim); the out-of-window partitions compute garbage that is never stored.
H_CHUNKS = [(0, 128, 0), (128, 72, 28)]
W_BLOCKS = [(0, 3), (3, 6), (9, 6), (15, 6), (21, 4)]
DATA_BUFS = 6

_cache: dict = {}


def _build_module():
    import concourse.bacc as bacc
    import concourse.mybir as mybir
    from concourse.tile import TileContext

    f32 = mybir.dt.float32
    u8 = mybir.dt.uint8

    nc = bacc.Bacc(trn_type="TRN2", debug=False, num_devices=NCORES)
    data = nc.dram_tensor("data", [CC, H, WS, N], f32, kind="ExternalInput")
    # hx pre-broadcast across partitions on host: [p, w, n]
    mxb = nc.dram_tensor("mxb", [128, WS, N], u8, kind="ExternalInput")
    # per-pixel not-inside mask, packed per h-chunk: [p, chunk, w, n]
    ninb = nc.dram_tensor("ninb", [128, 2, WS, N], u8, kind="ExternalInput")
    # hy packed per h-chunk: [p, chunk, n]
    myb = nc.dram_tensor("myb", [128, 2, N], u8, kind="ExternalInput")
    out = nc.dram_tensor("out", [H, WS, N], f32, kind="ExternalOutput")

    with TileContext(nc) as tc:
        with (
            tc.tile_pool(name="masks", bufs=1) as mpool,
            tc.tile_pool(name="dpool", bufs=DATA_BUFS) as dpool,
        ):
            zeros = mpool.tile([128, 1], f32)
            nc.vector.memset(zeros[:], 0.0)

            # y-masks packed in one small load: [128, 2, N]
            ymask = mpool.tile([128, 2, N], u8)
            nc.scalar.dma_start(ymask[:], myb[:])
            hy_t = [ymask[:, 0, :], ymask[:, 1, :]]

            # x-masks, already broadcast across partitions host-side;
            # loaded per w-block so the first compute isn't gated on the
            # whole mask tensor.
            xm_blocks = []
            for bi, (w0, wb) in enumerate(W_BLOCKS):
                t_xm = mpool.tile([128, wb, N], u8, tag=f"xm{bi}")
                nc.scalar.dma_start(t_xm[:], mxb[:, w0 : w0 + wb, :])
                xm_blocks.append(t_xm)

            for ci, (h0, ph, po) in enumerate(H_CHUNKS):
                sp = slice(po, po + ph)  # DMA partition window
                for bi, (w0, wb) in enumerate(W_BLOCKS):
                    # all 4 cell planes in one tile, loaded by ONE DMA so
                    # downstream ops wait on a single semaphore lane
                    dall = dpool.tile([128, CC, wb, N], f32, tag="dall")
                    nc.sync.dma_start(
                        dall[sp],
                        data[:, h0 : h0 + ph, w0 : w0 + wb, :].transpose(
                            [1, 0, 2, 3]
                        ),
                    )
                    t_nin = dpool.tile([128, wb, N], u8, tag="nin")
                    nc.scalar.dma_start(
                        t_nin[:], ninb[:, ci, w0 : w0 + wb, :]
                    )
                    hxv2 = xm_blocks[bi][:, None, :, :].broadcast_to(
                        (128, 2, wb, N)
                    )
                    hyv = hy_t[ci][:, None, :].broadcast_to((128, wb, N))
                    zv = zeros[:, :, None].broadcast_to((128, wb, N))
                    # x-blend both cell rows in one op, then y-blend, then zero
                    nc.vector.copy_predicated(
                        dall[:, 0::2], hxv2, dall[:, 1::2]
                    )
                    nc.vector.copy_predicated(dall[:, 0], hyv, dall[:, 2])
                    nc.vector.copy_predicated(dall[:, 0], t_nin[:], zv)
                    nc.scalar.dma_start(
                        out[h0 : h0 + ph, w0 : w0 + wb, :], dall[sp, 0]
                    )
    nc.finalize()
    return nc


def _get_module():
    if "nc" not in _cache:
        _cache["nc"] = _build_module()
    return _cache["nc"]


def _host_masks(rois):
    """Masks in f32 arithmetic bit-identical to the reference, as uint8."""
    r = np.asarray(rois, dtype=np.float32)
    x1, y1, x2, y2 = r[:, 0], r[:, 1], r[:, 2], r[:, 3]
    two = np.float32(2.0)
    one = np.float32(1.0)

    xs = np.arange(W, dtype=np.float32)[:, None]  # (W, 1)
    cw = np.maximum(x2 - x1, one)[None, :]  # (1, N)
    fx = np.floor(two * (xs - x1[None, :]) / cw)
    hx = (fx >= 1.0).astype(np.uint8)  # clip(floor, 0, 1) == 1
    nix = (~((xs >= x1[None, :]) & (xs <= x2[None, :]))).astype(np.uint8)

    ys = np.arange(H, dtype=np.float32)[:, None]  # (H, 1)
    ch = np.maximum(y2 - y1, one)[None, :]
    fy = np.floor(two * (ys - y1[None, :]) / ch)
    hy = (fy >= 1.0).astype(np.uint8)
    niy = (~((ys >= y1[None, :]) & (ys <= y2[None, :]))).astype(np.uint8)

    return hx, nix, hy, niy


def _run(data, rois, trace=False):
    from concourse.bass_utils import run_bass_kernel_spmd

    data = np.ascontiguousarray(np.asarray(data, dtype=np.float32))
    hx, nix, hy, niy = _host_masks(rois)

    # hy packed [128, 2, N]: chunk1 rows 0..127, chunk2 rows 128..199@28..100
    myb = np.zeros((128, 2, N), dtype=np.uint8)
    myb[:, 0] = hy[0:128]
    myb[28:100, 1] = hy[128:200]

    in_maps = []
    for i in range(NCORES):
        sl = slice(i * WS, (i + 1) * WS)
        mxb = np.ascontiguousarray(
            np.broadcast_to(hx[sl, :][None], (128, WS, N))
        )
        # not-inside per pixel: nix(w,n) OR niy(h,n), packed per h-chunk
        nin = np.maximum(nix[sl, :][None, :, :], niy[:, None, :])  # (H, WS, N)
        ninb = np.zeros((128, 2, WS, N), dtype=np.uint8)
        ninb[:, 0] = nin[0:128]
        ninb[28:100, 1] = nin[128:200]
        in_maps.append(
            {
                "data": np.ascontiguousarray(data[:, :, sl, :]),
                "mxb": mxb,
                "ninb": np.ascontiguousarray(ninb),
                "myb": myb,
            }
        )

    nc = _get_module()
    last_err = None
    for _attempt in range(2):
        try:
            res = run_bass_kernel_spmd(
                nc, in_maps, core_ids=list(range(NCORES)), trace=trace
            )
            break
        except Exception as e:  # transient NRT device errors: retry once
            last_err = e
    else:
        raise last_err
    full = np.concatenate([r["out"] for r in res.results], axis=1)
    return np.asarray(full, dtype=np.float32), res


def kernel(data, rois):
    out, _ = _run(data, rois, trace=False)
    return out



# revision 4
# speedup vs baseline: 1.0325x; 1.0325x over previous
"""Trainium2 Bass kernel for CropSplit (SipMask-style crop + quadrant split).

Reference computation, per output pixel (y, x, n):
    inside = point (x, y) lies in box rois[n] = (x1, y1, x2, y2)
    cell   = which of the 2x2 ROI sub-cells the pixel falls in
    out[y, x, n] = inside ? data[cell, y, x, n] : 0

Strategy (v2 — DMA-engine balanced):
  - Shard along W across the 8 cores (25 columns each); tile layout
    [h -> partitions, (cc, w, n) -> free] so every DMA row is a large
    contiguous DRAM block.
  - Each of the 16 SDMA engines serves a fixed group of 8 SBUF
    partitions, so per-engine bytes are set by how rows map to
    partitions.  H=200 rows are mapped:
      rows   0..127 -> partitions p = h          (stride 1: 8 rows/group)
      rows 128..191 -> partitions p = 2(h-128)   (stride 2: 4 rows/group)
      rows 192..199 -> partitions p = s + 16(h-192), s alternating 1/9
                       per w-block (1 row on even/odd groups alternately)
    => every engine group moves ~12.5 rows instead of the 8/16 split a
    naive 128+72 chunking gives (that imbalance capped the old kernel at
    ~75% DMA utilization in its second half).
  - Masks are computed on host in float32 with bit-identical arithmetic
    to the reference and shipped as uint8 packed in the same row layout:
        nin[p, hpart, w, n] = NOT inside   (zeroing mask)
        my[p, hpart, n]     = (cy == 1)    (quadrant row select)
    The x-mask hx[w, n] is identical on every partition, so only one
    partition's copy is shipped (5 KB) and gpsimd.partition_broadcast
    fans it out across all 128 partitions on-device (saves 640 KB of
    HBM traffic vs shipping it pre-broadcast).
  - Per tile, the 4-way select + mask is 3 predicated DVE ops:
        cp(dall[0::2], hx, dall[1::2])  (d0<-d1, d2<-d3 where cx==1)
        cp(dall[0], my, dall[2])        (y-blend -> 4-way select)
        cp(dall[0], nin, 0)             (zero outside the box)
    w-blocks are [7,7,6,4,1] (few big ops to amortize DVE op overhead,
    tiny last block so the drain tail is short).
  - All 4 cc planes of a tile arrive in one DMA; DMA issue is split
    across both HWDGE sequencers (Sync for data loads, Scalar for
    masks/stores).
"""

import numpy as np

C = 2
CC = C * C
H = W = N = 200
NCORES = 8
WS = W // NCORES  # 25 columns per core

# w-blocks: (w0, wb). Big first (amortize DVE op overhead), small last
# (short pipeline drain tail).
W_BLOCKS = [(0, 7), (7, 7), (14, 6), (20, 4), (24, 1)]
# per-block sigma for the 8-row tail (rows 192..199): partitions
# sigma+16j. Alternating 1/9 spreads the tail rows' bytes over even and
# odd engine groups so per-engine totals stay balanced.
SIGMA = [1, 9, 1, 9, 1]
DATA_BUFS = 5

_cache: dict = {}


def _row_of_partition():
    """row_map[block_parity][hpart][p] = source row h for partition p (or -1)."""
    maps = []
    for sigma in (1, 9):
        m = np.full((2, 128), -1, dtype=np.int64)
        m[0, :] = np.arange(128)  # hpart 0: rows 0..127
        m[1, 0:128:2] = 128 + np.arange(64)  # hpart 1: rows 128..191
        m[1, sigma:sigma + 16 * 8:16] = 192 + np.arange(8)  # rows 192..199
        maps.append(m)
    return maps


_ROWMAPS = _row_of_partition()


def _build_module():
    import concourse.bacc as bacc
    import concourse.mybir as mybir
    from concourse.tile import TileContext

    f32 = mybir.dt.float32
    u8 = mybir.dt.uint8

    nc = bacc.Bacc(trn_type="TRN2", debug=False, num_devices=NCORES)
    data = nc.dram_tensor("data", [CC, H, WS, N], f32, kind="ExternalInput")
    # hx (quadrant column select) for this core's 25 columns: one copy.
    mx1 = nc.dram_tensor("mx1", [1, WS, N], u8, kind="ExternalInput")
    # not-inside mask packed per (sigma-parity, hpart): [p, par, hpart, w, n]
    ninb = nc.dram_tensor("ninb", [128, 2, 2, WS, N], u8, kind="ExternalInput")
    # y-select mask packed the same way: [p, par, hpart, n]
    myb = nc.dram_tensor("myb", [128, 2, 2, N], u8, kind="ExternalInput")
    out = nc.dram_tensor("out", [H, WS, N], f32, kind="ExternalOutput")

    with TileContext(nc) as tc:
        with (
            tc.tile_pool(name="masks", bufs=1) as mpool,
            tc.tile_pool(name="dpool", bufs=DATA_BUFS) as dpool,
        ):
            zeros = mpool.tile([128, 1], f32)
            nc.vector.memset(zeros[:], 0.0)

            # y-masks in one small load: [128, 2, 2, N]
            ymask = mpool.tile([128, 2, 2, N], u8)
            nc.scalar.dma_start(ymask[:], myb[:])

            # x-mask: load one partition's copy, broadcast on gpsimd.
            mx_row = mpool.tile([1, WS, N], u8, tag="mxrow")
            nc.scalar.dma_start(mx_row[:], mx1[:])
            mxt = mpool.tile([128, WS, N], u8, tag="mxb")
            nc.gpsimd.partition_broadcast(mxt[:], mx_row[:])

            for bi, (w0, wb) in enumerate(W_BLOCKS):
                par = 0 if SIGMA[bi] == 1 else 1
                for hp in range(2):
                    # all 4 cell planes in one tile; loaded by one DMA
                    # (hp 0) or three DMAs into disjoint partition sets
                    # (hp 1) so every engine group stays fed.
                    dall = dpool.tile([128, CC, wb, N], f32, tag="dall")
                    src = data[:, :, w0 : w0 + wb, :]
                    if hp == 0:
                        nc.sync.dma_start(
                            dall[:],
                            src[:, 0:128].transpose([1, 0, 2, 3]),
                        )
                    else:
                        nc.sync.dma_start(
                            dall[0:128:2],
                            src[:, 128:192].transpose([1, 0, 2, 3]),
                        )
                        s = SIGMA[bi]
                        nc.sync.dma_start(
                            dall[s : s + 113 : 16],
                            src[:, 192:200].transpose([1, 0, 2, 3]),
                        )
                    t_nin = dpool.tile([128, wb, N], u8, tag="nin")
                    nc.scalar.dma_start(
                        t_nin[:], ninb[:, par, hp, w0 : w0 + wb, :]
                    )
                    hxv2 = mxt[:, None, w0 : w0 + wb, :].broadcast_to(
                        (128, 2, wb, N)
                    )
                    hyv = ymask[:, par, hp, None, :].broadcast_to((128, wb, N))
                    zv = zeros[:, :, None].broadcast_to((128, wb, N))
                    # x-blend both cell rows in one op, then y-blend, then zero
                    nc.vector.copy_predicated(
                        dall[:, 0::2], hxv2, dall[:, 1::2]
                    )
                    nc.vector.copy_predicated(dall[:, 0], hyv, dall[:, 2])
                    nc.vector.copy_predicated(dall[:, 0], t_nin[:], zv)
                    if hp == 0:
                        nc.scalar.dma_start(
                            out[0:128, w0 : w0 + wb, :], dall[0:128, 0]
                        )
                    else:
                        nc.scalar.dma_start(
                            out[128:192, w0 : w0 + wb, :], dall[0:128:2, 0]
                        )
                        s = SIGMA[bi]
                        nc.scalar.dma_start(
                            out[192:200, w0 : w0 + wb, :],
                            dall[s : s + 113 : 16, 0],
                        )
    nc.finalize()
    return nc


def _get_module():
    if "nc" not in _cache:
        _cache["nc"] = _build_module()
    return _cache["nc"]


def _host_masks(rois):
    """Masks in f32 arithmetic bit-identical to the reference, as uint8."""
    r = np.asarray(rois, dtype=np.float32)
    x1, y1, x2, y2 = r[:, 0], r[:, 1], r[:, 2], r[:, 3]
    two = np.float32(2.0)
    one = np.float32(1.0)

    xs = np.arange(W, dtype=np.float32)[:, None]  # (W, 1)
    cw = np.maximum(x2 - x1, one)[None, :]  # (1, N)
    fx = np.floor(two * (xs - x1[None, :]) / cw)
    hx = (fx >= 1.0).astype(np.uint8)  # clip(floor, 0, 1) == 1
    nix = (~((xs >= x1[None, :]) & (xs <= x2[None, :]))).astype(np.uint8)

    ys = np.arange(H, dtype=np.float32)[:, None]  # (H, 1)
    ch = np.maximum(y2 - y1, one)[None, :]
    fy = np.floor(two * (ys - y1[None, :]) / ch)
    hy = (fy >= 1.0).astype(np.uint8)
    niy = (~((ys >= y1[None, :]) & (ys <= y2[None, :]))).astype(np.uint8)

    return hx, nix, hy, niy


def _pack_rows(arr_by_row, fill):
    """arr_by_row: (H, ...) -> packed (128, 2, 2, ...) per (parity, hpart)."""
    shp = (128, 2, 2) + arr_by_row.shape[1:]
    outp = np.full(shp, fill, dtype=arr_by_row.dtype)
    for par in range(2):
        rm = _ROWMAPS[par]
        for hp in range(2):
            valid = rm[hp] >= 0
            outp[valid, par, hp] = arr_by_row[rm[hp][valid]]
    return outp


def _run(data, rois, trace=False):
    from concourse.bass_utils import run_bass_kernel_spmd

    data = np.ascontiguousarray(np.asarray(data, dtype=np.float32))
    hx, nix, hy, niy = _host_masks(rois)

    # y masks packed per (parity, hpart): [128, 2, 2, N]
    myb = _pack_rows(hy, 0)

    in_maps = []
    for i in range(NCORES):
        sl = slice(i * WS, (i + 1) * WS)
        # not-inside per pixel: nix(w,n) OR niy(h,n)  -> (H, WS, N)
        nin = np.maximum(nix[sl, :][None, :, :], niy[:, None, :])
        ninb = _pack_rows(nin, 1)
        in_maps.append(
            {
                "data": np.ascontiguousarray(data[:, :, sl, :]),
                "mx1": np.ascontiguousarray(hx[sl, :][None]),
                "ninb": np.ascontiguousarray(ninb),
                "myb": np.ascontiguousarray(myb),
            }
        )

    nc = _get_module()
    last_err = None
    for _attempt in range(2):
        try:
            res = run_bass_kernel_spmd(
                nc, in_maps, core_ids=list(range(NCORES)), trace=trace
            )
            break
        except Exception as e:  # transient NRT device errors: retry once
            last_err = e
    else:
        raise last_err
    full = np.concatenate([r["out"] for r in res.results], axis=1)
    return np.asarray(full, dtype=np.float32), res


def kernel(data, rois):
    out, _ = _run(data, rois, trace=False)
    return out
